# revision 17
# baseline (speedup 1.0000x reference)
"""Trainium2 Bass kernel for nn_LIIF_3d: Siren MLP over all pixels x 3 timestamps.

Math (from the reference): the nearest-neighbor grid sample at pixel-center
coords is the identity, so the whole op is
    out[t, b, :, p] = MLP([feat[b, :, p]; times[t]])
with a 65->64->64->256->256->256->64 Siren MLP, sin(30*z) activations.

Device strategy (per core, 8 cores, data-parallel over pixels):
  - channel-major activations: [channels(part), tokens(free)] tiles
  - fold the omega=30 scale into weights/biases on the host
  - the time channel is constant per timestamp -> fold w0[:,64]*t into the
    layer-0 bias; compute layer-0 pre-activation z0 once per token tile and
    reuse it for all 3 timestamps (different activation bias vectors)
  - matmuls in float32r (full-rate fp32 PE mode), activations fp32 on ACT
  - final bias-add on the vector engine to keep ACT (the bottleneck) lean
"""

import os
import sys

for _p in ("/opt/trn_rl_repo", "/root/.axon_site/_ro/trn_rl_repo"):
    if os.path.isdir(_p) and _p not in sys.path:
        sys.path.insert(0, _p)

import numpy as np

import concourse.bass as bass
import concourse.bacc as bacc
import concourse.mybir as mybir
from concourse.bass import ts
from concourse.tile import TileContext
from concourse.bass_utils import run_bass_kernel_spmd

F32 = mybir.dt.float32
F32R = mybir.dt.float32r
SIN = mybir.ActivationFunctionType.Sin

W0_SIREN = 30.0
B, C, H, W = 2, 64, 192, 320
QS = H * W                      # 61440 pixels per batch image
NCORES = 8
PPC = B * QS // NCORES          # 15360 pixels per core
TT = 1024                       # token tile (columns)
NT = PPC // TT                  # 15 tiles per core
NSUB = TT // 512                # matmul N-slices per tile

PI = float(np.pi)
TWO_PI = float(2 * np.pi)
INV2PI = float(1.0 / (2 * np.pi))
MAGIC = float(1.5 * 2**23)
RR_MODE = os.environ.get('BASS_RR', 'magic')
_MM_DT = {'f32': F32, 'f32r': F32R}[os.environ.get('BASS_MM', 'f32r')]


def _mm(v):
    return v




def _emit_sin(nc, rrp, pool_tag, h_out, z_in, bias_ap, bmod_ap, npi_ap, P, TT):
    """h_out = sin(z_in + bias) with range reduction on DVE."""
    if RR_MODE == 'mod2':
        r = rrp.tile([P, TT], F32, tag=pool_tag)
        nc.vector.tensor_scalar_add(r, z_in, bmod_ap)
        nc.vector.tensor_scalar(r, r, TWO_PI, None, mybir.AluOpType.mod)
        nc.scalar.activation(h_out, r, SIN, bias=npi_ap)
    else:
        u1 = rrp.tile([P, TT], F32, tag=pool_tag)
        nc.vector.tensor_scalar(u1, z_in, bias_ap, INV2PI,
                                mybir.AluOpType.add, mybir.AluOpType.mult)
        t = rrp.tile([P, TT], F32, tag=pool_tag + "t")
        nc.vector.tensor_scalar_add(t, u1, MAGIC)
        nc.vector.tensor_scalar_sub(t, t, MAGIC)
        nc.vector.tensor_sub(u1, u1, t)
        nc.scalar.activation(h_out, u1, SIN, scale=TWO_PI)

def _build_kernel():
    nc = bacc.Bacc("TRN2")

    x = nc.dram_tensor("x", [64, PPC], _MM_DT, kind="ExternalInput")
    wpk = nc.dram_tensor("wpk", [128, 1536], _MM_DT, kind="ExternalInput")
    bpk = nc.dram_tensor("bpk", [128, 22], F32, kind="ExternalInput")
    y = nc.dram_tensor("y", [3, 64, PPC], F32, kind="ExternalOutput")
    dbg = os.environ.get("BASS_DEBUG") == "1"
    if dbg:
        dbgw = nc.dram_tensor("dbgw", [128, 1536], _MM_DT, kind="ExternalOutput")
        dbgb = nc.dram_tensor("dbgb", [128, 22], F32, kind="ExternalOutput")
        dbgz0 = nc.dram_tensor("dbgz0", [64, TT], F32, kind="ExternalOutput")
        dbgh1 = nc.dram_tensor("dbgh1", [64, TT], _MM_DT, kind="ExternalOutput")
        dbgh2 = nc.dram_tensor("dbgh2", [64, TT], _MM_DT, kind="ExternalOutput")
        dbgh3 = nc.dram_tensor("dbgh3", [128, 2, TT], _MM_DT, kind="ExternalOutput")
        dbgh5 = nc.dram_tensor("dbgh5", [128, 2, TT], _MM_DT, kind="ExternalOutput")

    with TileContext(nc) as tc:
        with (
            tc.tile_pool(name="consts", bufs=1) as consts,
            tc.tile_pool(name="xin", bufs=3) as xin,
            tc.tile_pool(name="z0", bufs=2) as z0pool,
            tc.tile_pool(name="h64", bufs=3) as h64,
            tc.tile_pool(name="h256", bufs=3) as h256,
            tc.tile_pool(name="outp", bufs=4) as outp,
            tc.tile_pool(name="rr", bufs=3) as rrp,
            tc.tile_pool(name="ps", bufs=4, space="PSUM") as ps,
        ):
            # --- resident weights/biases (single packed DMA each) ------
            wp = consts.tile([128, 1536], _MM_DT, tag="wp")
            nc.sync.dma_start(wp, wpk[:, :])
            bp = consts.tile([128, 22], F32, tag="bp")
            nc.sync.dma_start(bp, bpk[:, :])
            w0s = wp[0:64, 0:64]
            w1s = wp[0:64, 64:128]
            w2s = wp[0:64, 128:384]
            w3s = [wp[:, 384:640], wp[:, 640:896]]
            w4s = [wp[:, 896:1152], wp[:, 1152:1408]]
            w5s = [wp[:, 1408:1472], wp[:, 1472:1536]]
            b0s = bp[0:64, 0:3]
            b1s = bp[0:64, 3:4]
            b2s = bp[:, 4:6]
            b3s = bp[:, 6:8]
            b4s = bp[:, 8:10]
            b5s = bp[0:64, 10:11]
            b0m = bp[0:64, 11:14]
            b1m = bp[0:64, 14:15]
            b2m = bp[:, 15:17]
            b3m = bp[:, 17:19]
            b4m = bp[:, 19:21]
            npi64 = bp[0:64, 21:22]
            npi128 = bp[:, 21:22]
            if dbg:
                nc.sync.dma_start(dbgw[:, :], wp)
                nc.sync.dma_start(dbgb[:, :], bp)

            # --- main loop over token tiles ----------------------------
            for it in range(NT):
                xt = xin.tile([64, TT], _MM_DT, tag="xt")
                nc.sync.dma_start(xt, x[:, ts(it, TT)])

                # z0 = W0' @ x  (shared by all 3 timestamps)
                z0p = ps.tile([64, TT], F32, tag="psA")
                for j in range(NSUB):
                    nc.tensor.matmul(
                        z0p[:, ts(j, 512)], _mm(w0s), _mm(xt[:, ts(j, 512)]),
                        start=True, stop=True,
                    )
                z0s = z0pool.tile([64, TT], F32, tag="z0s")
                nc.vector.tensor_copy(z0s, z0p)
                if dbg and it == 0:
                    nc.sync.dma_start(dbgz0[:, :], z0s)

                for c in range(3):
                    # L0 act: h1 = sin(z0 + b0'[c])
                    h1 = h64.tile([64, TT], _MM_DT, tag="h1")
                    _emit_sin(nc, rrp, "rr64", h1, z0s, b0s[:, c : c + 1],
                              b0m[:, c : c + 1], npi64, 64, TT)

                    if dbg and it == 0 and c == 0:
                        nc.sync.dma_start(dbgh1[:, :], h1)
                    # L1: 64 -> 64
                    p1 = ps.tile([64, TT], F32, tag="psA")
                    for j in range(NSUB):
                        nc.tensor.matmul(
                            p1[:, ts(j, 512)], _mm(w1s), _mm(h1[:, ts(j, 512)]),
                            start=True, stop=True,
                        )
                    h2 = h64.tile([64, TT], _MM_DT, tag="h2")
                    _emit_sin(nc, rrp, "rr64", h2, p1, b1s[:, 0:1],
                              b1m[:, 0:1], npi64, 64, TT)

                    if dbg and it == 0 and c == 0:
                        nc.sync.dma_start(dbgh2[:, :], h2)
                    # L2: 64 -> 256
                    h3 = h256.tile([128, 2, TT], _MM_DT, tag="h3")
                    for m in range(2):
                        p2 = ps.tile([128, TT], F32, tag="psA")
                        for j in range(NSUB):
                            nc.tensor.matmul(
                                p2[:, ts(j, 512)],
                                _mm(w2s[:, ts(m, 128)]),
                                _mm(h2[:, ts(j, 512)]),
                                start=True, stop=True,
                            )
                        _emit_sin(nc, rrp, "rr128", h3[:, m], p2, b2s[:, m : m + 1],
                                  b2m[:, m : m + 1], npi128, 128, TT)

                    if dbg and it == 0 and c == 0:
                        nc.sync.dma_start(dbgh3[:, :, :], h3)
                    # L3: 256 -> 256
                    h4 = h256.tile([128, 2, TT], _MM_DT, tag="h4")
                    for m in range(2):
                        p3 = ps.tile([128, TT], F32, tag="psA")
                        for j in range(NSUB):
                            for k in range(2):
                                nc.tensor.matmul(
                                    p3[:, ts(j, 512)],
                                    _mm(w3s[k][:, ts(m, 128)]),
                                    _mm(h3[:, k, ts(j, 512)]),
                                    start=(k == 0), stop=(k == 1),
                                )
                        _emit_sin(nc, rrp, "rr128", h4[:, m], p3, b3s[:, m : m + 1],
                                  b3m[:, m : m + 1], npi128, 128, TT)

                    # L4: 256 -> 256
                    h5 = h256.tile([128, 2, TT], _MM_DT, tag="h5")
                    for m in range(2):
                        p4 = ps.tile([128, TT], F32, tag="psA")
                        for j in range(NSUB):
                            for k in range(2):
                                nc.tensor.matmul(
                                    p4[:, ts(j, 512)],
                                    _mm(w4s[k][:, ts(m, 128)]),
                                    _mm(h5_in := h4[:, k, ts(j, 512)]),
                                    start=(k == 0), stop=(k == 1),
                                )
                        _emit_sin(nc, rrp, "rr128", h5[:, m], p4, b4s[:, m : m + 1],
                                  b4m[:, m : m + 1], npi128, 128, TT)

                    if dbg and it == 0 and c == 0:
                        nc.sync.dma_start(dbgh5[:, :, :], h5)
                    # L5: 256 -> 64 (no sin; bias on vector engine)
                    p5 = ps.tile([64, TT], F32, tag="psA")
                    for j in range(NSUB):
                        for k in range(2):
                            nc.tensor.matmul(
                                p5[:, ts(j, 512)],
                                _mm(w5s[k]),
                                _mm(h5[:, k, ts(j, 512)]),
                                start=(k == 0), stop=(k == 1),
                            )
                    ot = outp.tile([64, TT], F32, tag="ot")
                    nc.vector.tensor_scalar_add(ot, p5, b5s[:, 0:1])
                    nc.sync.dma_start(y[c, :, ts(it, TT)], ot)

    return nc


_NC_CACHE = None


def _get_nc():
    global _NC_CACHE
    if _NC_CACHE is None:
        _NC_CACHE = _build_kernel()
        _NC_CACHE.finalize()
    return _NC_CACHE


def kernel(feat, times, w0, b0, w1, b1, w2, b2, w3, b3, w4, b4, w5, b5,
           _trace=False, _trace_kwargs=None):
    feat = np.asarray(feat, np.float32)
    times = np.asarray(times, np.float32)

    s = np.float32(W0_SIREN)
    # host-side prep: transpose to [in, out], fold omega into w/b
    wt0 = np.ascontiguousarray((s * w0[:, :64]).T)        # [64, 64]
    b0t = np.ascontiguousarray(
        s * (b0[:, None] + w0[:, 64:65] * times[None, :].astype(np.float32))
    ).astype(np.float32)                                   # [64, 3]
    wt1 = np.ascontiguousarray((s * w1).T)                 # [64, 64]
    b1c = np.ascontiguousarray((s * b1)[:, None])          # [64, 1]
    wt2 = np.ascontiguousarray((s * w2).T)                 # [64, 256]
    b2c = np.ascontiguousarray((s * b2).reshape(2, 128).T)  # [128, 2]
    wt3 = np.ascontiguousarray((s * w3).T)                 # [256, 256]
    b3c = np.ascontiguousarray((s * b3).reshape(2, 128).T)
    wt4 = np.ascontiguousarray((s * w4).T)
    b4c = np.ascontiguousarray((s * b4).reshape(2, 128).T)
    wt5 = np.ascontiguousarray(w5.T)                       # [256, 64]
    b5c = np.ascontiguousarray(b5[:, None])                # [64, 1]

    wpk = np.zeros((128, 1536), np.float32)
    wpk[0:64, 0:64] = wt0
    wpk[0:64, 64:128] = wt1
    wpk[0:64, 128:384] = wt2
    wpk[:, 384:640] = wt3[0:128]
    wpk[:, 640:896] = wt3[128:256]
    wpk[:, 896:1152] = wt4[0:128]
    wpk[:, 1152:1408] = wt4[128:256]
    wpk[:, 1408:1472] = wt5[0:128]
    wpk[:, 1472:1536] = wt5[128:256]
    bpk = np.zeros((128, 22), np.float32)
    bpk[0:64, 0:3] = b0t
    bpk[0:64, 3:4] = b1c
    bpk[:, 4:6] = b2c
    bpk[:, 6:8] = b3c
    bpk[:, 8:10] = b4c
    bpk[0:64, 10:11] = b5c
    off = np.float32(33 * np.pi)
    bpk[0:64, 11:14] = b0t + off
    bpk[0:64, 14:15] = b1c + off
    bpk[:, 15:17] = b2c + off
    bpk[:, 17:19] = b3c + off
    bpk[:, 19:21] = b4c + off
    bpk[:, 21] = -np.pi

    flat = np.asarray(feat, np.float32).reshape(B, C, QS)
    shared = dict(wpk=wpk, bpk=bpk)
    in_maps = []
    for core in range(NCORES):
        b_idx = core // (NCORES // B)
        chunk = core % (NCORES // B)
        p0 = chunk * PPC
        x_c = np.ascontiguousarray(flat[b_idx, :, p0 : p0 + PPC])
        in_maps.append({"x": x_c, **shared})

    nc = _get_nc()
    kw = {}
    if _trace:
        kw = dict(trace=True, trace_kwargs=_trace_kwargs or {})
    try:
        res = run_bass_kernel_spmd(nc, in_maps, list(range(NCORES)), **kw)
    except Exception:
        res = run_bass_kernel_spmd(nc, in_maps, list(range(NCORES)), **kw)

    out = np.empty((3, B, C, QS), np.float32)
    for core in range(NCORES):
        b_idx = core // (NCORES // B)
        chunk = core % (NCORES // B)
        p0 = chunk * PPC
        out[:, b_idx, :, p0 : p0 + PPC] = res.results[core]["y"]
    out = out.reshape(3, B, C, H, W)
    if _trace:
        return out, res
    return out
